# revision 2
# baseline (speedup 1.0000x reference)
"""Trainium2 Bass kernel for nn_EncoderInput (DA-RNN input-attention encoder).

Data-parallel over batch: 8 cores x 16 batch rows each. Full inputs in,
full output out; sharding/marshalling (transposes, bf16 casts, chunk
re-layout) happens host-side, all FLOPs happen on-device.

Per-core algorithm (B=16 shard, T=128 steps, N=128 drivers, M=256 hidden):
  UX[b,n,u]   = sum_t X[b,t,n] Ud[t,u] + bU[u] + bW[u]   (precomputed, PE)
  XW[t,m,b]   = (x_t @ Wx + b)^T                          (precomputed, PE)
  per step t (state: gz tile = [i~ f~ o~ g~ | c], H = 2h):
    z^T  = XW[t] + (Wh/2)^T-chunks @ H^T    (PE)
    gz   = tanh(z/2)  (i~,f~,o~ halves of sigmoid; g~ = tanh, w x2)  (ACT)
    u    = (gz[i,f]+1)*(gz[g],c) = [2ig | 2fc]            (DVE STT, 1 op)
    s2   = u_a + u_b = 2 c_new                            (DVE TT)
    c    = 0.5*s2 -> next gz c-slot                       (DVE TS)
    tc   = tanh(0.5*s2) = tanh(c_new)                     (ACT, scale trick)
    H    = (o~+1)*tc = 2h                                 (DVE STT)
    w^T  = (Wd-scaled)^T-chunks @ [H;c]^T                 (PE into z psum)
    arg  = UX^T + w^T[:,b] broadcast                      (DVE TT, lag 2)
    TANH = tanh(arg)   one [128,2048] op                  (ACT, lag 4)
    e    = sparse-vd matmuls -> psum slots p=8b+s         (PE, lag 6)
  per 8-step group: exp(+accum sum) ; alpha*X fused STT ; DMA out
"""

import sys

for _p in ("/opt/trn_rl_repo",):
    if _p not in sys.path:
        sys.path.insert(0, _p)

import numpy as np
import ml_dtypes

BF16 = ml_dtypes.bfloat16

import concourse.bass as bass
import concourse.tile as tile
from concourse import bacc, mybir

F32 = mybir.dt.float32
BF = mybir.dt.bfloat16
AF = mybir.ActivationFunctionType
ALU = mybir.AluOpType
AX = mybir.AxisListType

B, T, N, M = 128, 128, 128, 256
NCORES = 8
BS = B // NCORES          # 16 batch rows per core
G = 8                     # steps per softmax group
NG = T // G               # 16 groups
# psum slot -> m-chunk of z; layout becomes [i0 i1 f0 f1 o0 o1 g0 g1]
PERM = [0, 1, 2, 3, 6, 7, 4, 5]

# software-pipeline lags (in emission windows)
LAG_W = 2      # w-broadcast of step t emitted in window t+LAG_W
LAG_TANH = 4   # big tanh of step t in window t+LAG_TANH
LAG_E = 6      # e-matmuls of step t in window t+LAG_E
LAG_SM = 15    # softmax tail of group g at window 8g+LAG_SM


def _build_kernel(nc):
    x_d = nc.dram_tensor("x", [BS, T, N], F32, kind="ExternalInput")
    xnb_d = nc.dram_tensor("xnb", [T, BS, N], BF, kind="ExternalInput")
    xtb_d = nc.dram_tensor("xtb", [N, T, BS], BF, kind="ExternalInput")
    wht_d = nc.dram_tensor("wht", [128, 2, 8, 128], BF, kind="ExternalInput")
    wxt_d = nc.dram_tensor("wxt", [128, 8, 128], BF, kind="ExternalInput")
    udt_d = nc.dram_tensor("udt", [T, T], BF, kind="ExternalInput")
    wdt_d = nc.dram_tensor("wdt", [128, 4, T], BF, kind="ExternalInput")
    s_d = nc.dram_tensor("svd", [T, 32, 32], BF, kind="ExternalInput")
    bt_d = nc.dram_tensor("bt", [128, 8], F32, kind="ExternalInput")
    bwc_d = nc.dram_tensor("bwc", [T, 1], F32, kind="ExternalInput")
    buc_d = nc.dram_tensor("buc", [T, 1], F32, kind="ExternalInput")
    h0t_d = nc.dram_tensor("h0t", [128, 32], BF, kind="ExternalInput")
    c0t_d = nc.dram_tensor("c0t", [128, 32], BF, kind="ExternalInput")
    eye_d = nc.dram_tensor("eye", [128, 128], BF, kind="ExternalInput")
    out_d = nc.dram_tensor("out", [BS, T, N], F32, kind="ExternalOutput")

    with tile.TileContext(nc) as tc:
        with tc.tile_pool(name="const", bufs=1) as const:
            # persistent SBUF residents
            uxt = const.tile([128, N, BS], BF)         # [u, n, b]
            xwt = const.tile([128, 8, T, BS], BF)      # [m, slot, t, b]
            wht = const.tile([128, 2, 8, 128], BF)
            wxt = const.tile([128, 8, 128], BF)
            udt = const.tile([T, T], BF)
            wdt = const.tile([128, 4, T], BF)
            svd = const.tile([T, 32, 32], BF)
            bt = const.tile([128, 8], F32)
            bwc = const.tile([T, 1], F32)
            buc = const.tile([T, 1], F32)
            eye = const.tile([128, 128], BF)
            bubw = const.tile([T, 1], F32)
            xnb = const.tile([T, BS, N], BF)
            xtb = const.tile([N, T, BS], BF)
            xga = const.tile([128, 16, N], F32)
            h0 = const.tile([128, 32], BF)

            for sb, dr in [
                (wht, wht_d), (wxt, wxt_d), (udt, udt_d), (wdt, wdt_d),
                (svd, s_d), (bt, bt_d), (bwc, bwc_d), (buc, buc_d),
                (eye, eye_d), (xnb, xnb_d), (xtb, xtb_d),
                (h0, h0t_d),
            ]:
                nc.sync.dma_start(out=sb[:], in_=dr.ap())

            for bb in range(BS):
                src_ap = bass.AP(
                    tensor=x_d, offset=bb * T * N,
                    ap=[[N, 8], [8 * N, 16], [1, N]])
                nc.sync.dma_start(out=xga[8 * bb:8 * bb + 8, :, :],
                                  in_=src_ap)

            nc.vector.tensor_add(bubw[:], buc[:], bwc[:])

            # ---------------- precompute ----------------
            with tc.tile_pool(name="pre", bufs=2, space="PSUM") as pre:
                # UX^T: [u,(b,n)] = sum_t Ud[t,u] * X[b,t,n]  (+ bU + bW)
                for q in range(2):
                    ps = pre.tile([128, 1024], F32)
                    for r in range(2):
                        idx = 2 * q + r
                        nc.tensor.matmul(
                            ps[:, 512 * r:512 * (r + 1)],
                            udt[:],
                            xnb[:, 4 * idx:4 * (idx + 1), :],
                            start=True, stop=True,
                        )
                    uxt_dst = bass.AP(
                        tensor=uxt.tensor, offset=uxt.offset + 8 * q,
                        ap=[uxt.ap[0], [1, 8], [BS, N]])
                    nc.scalar.activation(
                        uxt_dst, ps[:].rearrange("p (b n) -> p b n", b=8),
                        AF.Identity, bias=bubw[:, 0:1],
                    )
                # XW^T: [m,(t,b)] = sum_n Wx[n,m] X[b,t,n]  (+ b)
                for sl in range(8):
                    ps = pre.tile([128, 1024], F32)
                    for q in range(2):
                        for r in range(2):
                            tq = 32 * (2 * q + r)
                            nc.tensor.matmul(
                                ps[:, 512 * r:512 * (r + 1)],
                                wxt[:, sl, :],
                                xtb[:, tq:tq + 32, :],
                                start=True, stop=True,
                            )
                        dst = xwt[:, sl, 64 * q:64 * (q + 1), :]
                        if sl % 2 == 0:
                            nc.scalar.activation(
                                dst, ps[:], AF.Identity, bias=bt[:, sl:sl + 1])
                        else:
                            nc.vector.tensor_scalar(
                                dst, ps[:], bt[:, sl:sl + 1], None, ALU.add)

            # ---------------- main loop ----------------
            with (
                tc.tile_pool(name="zps", bufs=3, space="PSUM") as zps,
                tc.tile_pool(name="eps", bufs=2, space="PSUM") as eps,
                tc.tile_pool(name="state", bufs=4) as state,
                tc.tile_pool(name="hpool", bufs=4) as hpool,
                tc.tile_pool(name="work", bufs=4) as work,
                tc.tile_pool(name="wsb", bufs=4) as wsb,
                tc.tile_pool(name="argp", bufs=4) as argp,
                tc.tile_pool(name="thp", bufs=4) as thp,
                tc.tile_pool(name="soft", bufs=2) as soft,
            ):
                H_of = {0: h0}
                gz_of = {}
                w_of = {}     # step -> wT sbuf tile (128, BS)
                arg_of = {}   # step -> ARG tile
                tanh_of = {}  # step -> TANH tile
                e_of = {}     # group -> E psum tile

                # bootstrap: gz tile for step 0 holds c0 in its c-slot
                gz0 = state.tile([128, 160], BF, tag="gz")
                nc.sync.dma_start(out=gz0[:, 128:160], in_=c0t_d.ap())
                gz_of[0] = gz0

                for t in range(T + LAG_SM + 1):
                    if t <= T - 1:
                        # --- PE: z matmuls of step t ---
                        hp = tc.high_priority(offset=400)
                        hp.__enter__()
                        zt = zps.tile([128, 144], F32, tag="zt")
                        # single full-bank matmul: copies XW for all 8 slots
                        # into PSUM and clears the bank (start=True)
                        nc.tensor.matmul(
                            zt[:, 0:128], eye[:], xwt[:, :, t, :],
                            start=True, stop=False)
                        for sl in range(8):
                            for kc in range(2):
                                nc.tensor.matmul(
                                    zt[:, 16 * sl:16 * (sl + 1)],
                                    wht[:, kc, sl, :],
                                    H_of[t][:, 16 * kc:16 * (kc + 1)],
                                    start=False, stop=(kc == 1))

                        gz = gz_of[t]
                        gz_next = state.tile([128, 160], BF, tag="gz")
                        gz_of[t + 1] = gz_next

                        # --- ACT: gates (one instr; g-weights pre-scaled) ---
                        nc.scalar.activation(gz[:, 0:128], zt[:, 0:128],
                                             AF.Tanh, scale=0.5)

                        # --- DVE: fused LSTM update ---
                        # u = [(i~+1)*g~ | (f~+1)*c] = [2ig | 2fc]
                        u = work.tile([128, 64], BF, tag="u")
                        nc.vector.scalar_tensor_tensor(
                            u[:], gz[:, 0:64], 1.0, gz[:, 96:160],
                            ALU.add, ALU.mult)
                        s2 = work.tile([128, 32], F32, tag="s2")
                        nc.vector.tensor_tensor(s2[:], u[:, 0:32], u[:, 32:64],
                                                ALU.add)
                        # c_new = 0.5*s2 -> c-slot of next gz
                        nc.vector.tensor_scalar(gz_next[:, 128:160], s2[:],
                                                0.5, None, ALU.mult)
                        # tanh(c_new) = tanh(0.5*s2)
                        tct = work.tile([128, 32], BF, tag="tc")
                        nc.scalar.activation(tct[:], s2[:], AF.Tanh, scale=0.5)
                        # H = (o~+1)*tc = 2h
                        Hn = hpool.tile([128, 32], BF, tag="H")
                        nc.vector.scalar_tensor_tensor(
                            Hn[:], gz[:, 64:96], 1.0, tct[:],
                            ALU.add, ALU.mult)
                        hp.__exit__(None, None, None)
                        H_of[t + 1] = Hn
                        H_of.pop(t, None)

                        # --- PE: w^T matmuls (into ZWW region) ---
                        hs = [Hn[:, 0:16], Hn[:, 16:32],
                              gz_next[:, 128:144], gz_next[:, 144:160]]
                        for kc in range(4):
                            nc.tensor.matmul(
                                zt[:, 128:144], wdt[:, kc, :], hs[kc],
                                start=False, stop=(kc == 3))
                        wt_sb = wsb.tile([128, BS], BF, tag="wt")
                        nc.vector.tensor_copy(wt_sb[:], zt[:, 128:144])
                        w_of[t] = wt_sb
                        gz_of.pop(t, None)

                    # --- broadcast w over n for step t-LAG_W ---
                    tw = t - LAG_W
                    if 0 <= tw <= T - 1:
                        wt_sb = w_of.pop(tw)
                        arg = argp.tile([128, N, BS], BF, tag="arg")
                        wt_bc = bass.AP(
                            tensor=wt_sb.tensor, offset=wt_sb.offset,
                            ap=[wt_sb.ap[0], [0, N], [1, BS]])
                        nc.vector.tensor_tensor(
                            arg[:], uxt[:], wt_bc, ALU.add)
                        arg_of[tw] = arg

                    # --- ACT: big tanh of step t-LAG_TANH (one instr) ---
                    tt = t - LAG_TANH
                    if tt >= 0 and tt in arg_of:
                        th = thp.tile([128, N, BS], BF, tag="th")
                        nc.scalar.activation(th[:], arg_of.pop(tt)[:], AF.Tanh)
                        tanh_of[tt] = th

                    # --- PE: e-matmuls of step t-LAG_E ---
                    te = t - LAG_E
                    if te >= 0 and te in tanh_of:
                        se, ge = te % G, te // G
                        if ge not in e_of:
                            e_of[ge] = eps.tile([128, N], F32, tag="eps",
                                                name="etile")
                            nc.vector.memset(e_of[ge][:], 0.0)
                        ep = e_of[ge]
                        th = tanh_of.pop(te)
                        for bb in range(BS):
                            cg = bb // 4
                            v = (bb % 4) * 8 + se
                            nc.tensor.matmul(
                                ep[32 * cg:32 * (cg + 1), :],
                                svd[:, v, :],
                                th[:, :, bb],
                                start=False,
                                stop=(se == G - 1 and bb % 4 == 3),
                                tile_position=(0, 32 * cg),
                            )

                    # --- softmax + output of group (t-LAG_SM)//G ---
                    if t >= LAG_SM and (t - LAG_SM) % G == 0:
                        gs = (t - LAG_SM) // G
                        if gs in e_of:
                            ep = e_of.pop(gs)
                            ex = soft.tile([128, N], F32, tag="ex")
                            sm = soft.tile([128, 1], F32, tag="sm")
                            nc.scalar.activation(ex[:], ep[:], AF.Exp,
                                                 accum_out=sm[:])
                            rc = soft.tile([128, 1], F32, tag="rc")
                            nc.vector.reciprocal(rc[:], sm[:])
                            # out = (exp * 1/sum) * X  in one fused op
                            ot = soft.tile([128, N], F32, tag="ot")
                            nc.vector.scalar_tensor_tensor(
                                ot[:], ex[:], rc[:, 0:1], xga[:, gs, :],
                                ALU.mult, ALU.mult)
                            nc.sync.dma_start(
                                out=out_d.ap()[:, G * gs:G * (gs + 1), :],
                                in_=ot[:])
    return nc


_CACHE = {}


def _get_nc():
    if "nc" not in _CACHE:
        nc = bacc.Bacc("TRN2", target_bir_lowering=False, debug=False)
        _build_kernel(nc)
        nc.compile()
        _CACHE["nc"] = nc
    return _CACHE["nc"]


def kernel(X, h0, s0, Wx, Wh, b, Wd, bW, Ud, bU, vd, bv):
    X = np.asarray(X, np.float32)
    h0 = np.asarray(h0, np.float32)
    s0 = np.asarray(s0, np.float32)
    Wx = np.asarray(Wx, np.float32)
    Wh = np.asarray(Wh, np.float32)
    b = np.asarray(b, np.float32)
    Wd = np.asarray(Wd, np.float32)
    bW = np.asarray(bW, np.float32)
    Ud = np.asarray(Ud, np.float32)
    bU = np.asarray(bU, np.float32)
    vd = np.asarray(vd, np.float32)

    # replicated (weight) marshalling — layout only, no FLOPs
    # Wh scaled by 0.5 (state H = 2h); g-gate slots re-doubled so the
    # single tanh(0.5*z) gate activation yields tanh(z_g) for g.
    wht = np.ascontiguousarray(
        Wh.reshape(2, 128, 8, 128).transpose(1, 0, 2, 3)[:, :, PERM, :]
    ).astype(np.float32) * 0.5
    wht[:, :, 6:8, :] *= 2.0
    wht = wht.astype(BF16)
    wxt = np.ascontiguousarray(
        Wx.reshape(128, 8, 128)[:, PERM, :]).astype(np.float32)
    wxt[:, 6:8, :] *= 2.0
    wxt = wxt.astype(BF16)
    udt = Ud.astype(BF16)
    # Wd h-half scaled by 0.5 (H = 2h); c-half unscaled
    wdt = np.ascontiguousarray(Wd.reshape(4, 128, 128).transpose(1, 0, 2)
                               ).astype(np.float32)
    wdt[:, 0:2, :] *= 0.5
    wdt = wdt.astype(BF16)
    svd = np.zeros((128, 32, 32), np.float32)
    for v in range(32):
        svd[:, v, v] = vd[:, 0]
    svd = svd.astype(BF16)
    bt = np.ascontiguousarray(b.reshape(8, 128)[PERM].T).astype(np.float32)
    bt[:, 6:8] *= 2.0
    bwc = bW.reshape(T, 1).astype(np.float32)
    buc = bU.reshape(T, 1).astype(np.float32)
    eye = np.eye(128, dtype=BF16)

    def tr_state(v):  # (16,256) -> (128, 32) with col = 16*j + b
        return np.ascontiguousarray(
            v.T.reshape(2, 128, BS).transpose(1, 0, 2).reshape(128, 2 * BS))

    in_maps = []
    for c in range(NCORES):
        xs = X[BS * c:BS * (c + 1)]
        in_maps.append({
            "x": np.ascontiguousarray(xs),
            "xnb": np.ascontiguousarray(xs.transpose(1, 0, 2)).astype(BF16),
            "xtb": np.ascontiguousarray(xs.transpose(2, 1, 0)).astype(BF16),
            "wht": wht, "wxt": wxt, "udt": udt, "wdt": wdt, "svd": svd,
            "bt": bt, "bwc": bwc, "buc": buc, "eye": eye,
            "h0t": (tr_state(h0[BS * c:BS * (c + 1)]) * 2.0).astype(BF16),
            "c0t": tr_state(s0[BS * c:BS * (c + 1)]).astype(BF16),
        })

    from concourse.bass_utils import run_bass_kernel_spmd
    nc = _get_nc()
    _CACHE["in_maps"] = in_maps
    res = run_bass_kernel_spmd(nc, in_maps, core_ids=list(range(NCORES)))
    out = np.concatenate(
        [np.asarray(res.results[c]["out"]) for c in range(NCORES)], axis=0)
    return out.astype(np.float32)


# revision 3
# speedup vs baseline: 1.6818x; 1.6818x over previous
"""Trainium2 Bass kernel for nn_EncoderInput (DA-RNN input-attention encoder).

Data-parallel over batch: 8 cores x 16 batch rows each. Full inputs in,
full output out; sharding/marshalling (transposes, bf16 casts, chunk
re-layout) happens host-side, all FLOPs happen on-device.

Per-core algorithm (B=16 shard, T=128 steps, N=128 drivers, M=256 hidden):
  UX[b,n,u]   = sum_t X[b,t,n] Ud[t,u] + bU[u] + bW[u]   (precomputed, PE)
  XW[t,m,b]   = (x_t @ Wx + b)^T                          (precomputed, PE)
  per step t (state: gz tile = [i~ f~ o~ g~ | c], H = 2h):
    z^T  = XW[t] + (Wh/2)^T-chunks @ H^T    (PE)
    gz   = tanh(z/2)  (i~,f~,o~ halves of sigmoid; g~ = tanh, w x2)  (ACT)
    u    = (gz[i,f]+1)*(gz[g],c) = [2ig | 2fc]            (DVE STT, 1 op)
    s2   = u_a + u_b = 2 c_new                            (DVE TT)
    c    = 0.5*s2 -> next gz c-slot                       (DVE TS)
    tc   = tanh(0.5*s2) = tanh(c_new)                     (ACT, scale trick)
    H    = (o~+1)*tc = 2h                                 (DVE STT)
    w^T  = (Wd-scaled)^T-chunks @ [H;c]^T                 (PE into z psum)
    arg  = UX^T + w^T[:,b] broadcast                      (DVE TT, lag 2)
    TANH = tanh(arg)  two [128,1024] halves packed around (ACT, lag 4)
          the recurrence's gates/tanh_c ops in the ACT queue
    e    = sparse-vd matmuls -> psum slots p=8b+s         (PE, lag 6)
  per 8-step group: exp(+accum sum) ; alpha*X fused STT ; DMA out

Every compute engine is given an explicit total order via no-sync
scheduling edges so the FIFO queues pack tightly without head-of-line
blocking of the recurrence by the big attention tanh.
"""

import sys

for _p in ("/opt/trn_rl_repo",):
    if _p not in sys.path:
        sys.path.insert(0, _p)

import numpy as np
import ml_dtypes

BF16 = ml_dtypes.bfloat16

import concourse.bass as bass
import concourse.tile as tile
from concourse.tile import add_dep_helper
from concourse import bacc, mybir

F32 = mybir.dt.float32
BF = mybir.dt.bfloat16
AF = mybir.ActivationFunctionType
ALU = mybir.AluOpType
AX = mybir.AxisListType

B, T, N, M = 128, 128, 128, 256
NCORES = 8
BS = B // NCORES          # 16 batch rows per core
G = 8                     # steps per softmax group
NG = T // G               # 16 groups
# psum slot -> m-chunk of z; layout becomes [i0 i1 f0 f1 o0 o1 g0 g1]
PERM = [0, 1, 2, 3, 6, 7, 4, 5]

# software-pipeline lags (in emission windows)
LAG_W = 2      # w-broadcast of step t emitted in window t+LAG_W
LAG_TANH = 4   # big tanh halves of step t in window t+LAG_TANH
LAG_E = 6      # e-matmuls of step t in window t+LAG_E
LAG_SM = 15    # softmax tail of group g at window 8g+LAG_SM


def _build_kernel(nc):
    x_d = nc.dram_tensor("x", [BS, T, N], F32, kind="ExternalInput")
    xnb_d = nc.dram_tensor("xnb", [T, BS, N], BF, kind="ExternalInput")
    xtb_d = nc.dram_tensor("xtb", [N, T, BS], BF, kind="ExternalInput")
    wht_d = nc.dram_tensor("wht", [128, 2, 8, 128], BF, kind="ExternalInput")
    wxt_d = nc.dram_tensor("wxt", [128, 8, 128], BF, kind="ExternalInput")
    udt_d = nc.dram_tensor("udt", [T, T], BF, kind="ExternalInput")
    wdt_d = nc.dram_tensor("wdt", [128, 4, T], BF, kind="ExternalInput")
    s_d = nc.dram_tensor("svd", [T, 32, 32], BF, kind="ExternalInput")
    bt_d = nc.dram_tensor("bt", [128, 8], F32, kind="ExternalInput")
    bwc_d = nc.dram_tensor("bwc", [T, 1], F32, kind="ExternalInput")
    buc_d = nc.dram_tensor("buc", [T, 1], F32, kind="ExternalInput")
    h0t_d = nc.dram_tensor("h0t", [128, 32], BF, kind="ExternalInput")
    c0t_d = nc.dram_tensor("c0t", [128, 32], BF, kind="ExternalInput")
    eye_d = nc.dram_tensor("eye", [128, 128], BF, kind="ExternalInput")
    out_d = nc.dram_tensor("out", [BS, T, N], F32, kind="ExternalOutput")

    # per-engine total-order chain (no-sync scheduling edges)
    last = {}

    def chain(key, instr):
        if key in last:
            add_dep_helper(instr.ins, last[key].ins, sync=False,
                           reason="engine order")
        last[key] = instr
        return instr

    with tile.TileContext(nc) as tc:
        with tc.tile_pool(name="const", bufs=1) as const:
            # persistent SBUF residents
            uxt = const.tile([128, N, BS], BF)         # [u, n, b]
            xwt = const.tile([128, 8, T, BS], BF)      # [m, slot, t, b]
            wht = const.tile([128, 2, 8, 128], BF)
            wxt = const.tile([128, 8, 128], BF)
            udt = const.tile([T, T], BF)
            wdt = const.tile([128, 4, T], BF)
            svd = const.tile([T, 32, 32], BF)
            bt = const.tile([128, 8], F32)
            bwc = const.tile([T, 1], F32)
            buc = const.tile([T, 1], F32)
            eye = const.tile([128, 128], BF)
            bubw = const.tile([T, 1], F32)
            xnb = const.tile([T, BS, N], BF)
            xtb = const.tile([N, T, BS], BF)
            xga = const.tile([128, 16, N], F32)
            h0 = const.tile([128, 32], BF)

            for sb, dr in [
                (wht, wht_d), (wxt, wxt_d), (udt, udt_d), (wdt, wdt_d),
                (svd, s_d), (bt, bt_d), (bwc, bwc_d), (buc, buc_d),
                (eye, eye_d), (xnb, xnb_d), (xtb, xtb_d),
                (h0, h0t_d),
            ]:
                nc.sync.dma_start(out=sb[:], in_=dr.ap())

            for bb in range(BS):
                src_ap = bass.AP(
                    tensor=x_d, offset=bb * T * N,
                    ap=[[N, 8], [8 * N, 16], [1, N]])
                nc.sync.dma_start(out=xga[8 * bb:8 * bb + 8, :, :],
                                  in_=src_ap)

            nc.vector.tensor_add(bubw[:], buc[:], bwc[:])

            # ---------------- precompute ----------------
            with tc.tile_pool(name="pre", bufs=2, space="PSUM") as pre:
                # UX^T: [u,(b,n)] = sum_t Ud[t,u] * X[b,t,n]  (+ bU + bW)
                for q in range(2):
                    ps = pre.tile([128, 1024], F32)
                    for r in range(2):
                        idx = 2 * q + r
                        nc.tensor.matmul(
                            ps[:, 512 * r:512 * (r + 1)],
                            udt[:],
                            xnb[:, 4 * idx:4 * (idx + 1), :],
                            start=True, stop=True,
                        )
                    uxt_dst = bass.AP(
                        tensor=uxt.tensor, offset=uxt.offset + 8 * q,
                        ap=[uxt.ap[0], [1, 8], [BS, N]])
                    nc.scalar.activation(
                        uxt_dst, ps[:].rearrange("p (b n) -> p b n", b=8),
                        AF.Identity, bias=bubw[:, 0:1],
                    )
                # XW^T: [m,(t,b)] = sum_n Wx[n,m] X[b,t,n]  (+ b)
                for sl in range(8):
                    ps = pre.tile([128, 1024], F32)
                    for q in range(2):
                        for r in range(2):
                            tq = 32 * (2 * q + r)
                            nc.tensor.matmul(
                                ps[:, 512 * r:512 * (r + 1)],
                                wxt[:, sl, :],
                                xtb[:, tq:tq + 32, :],
                                start=True, stop=True,
                            )
                        dst = xwt[:, sl, 64 * q:64 * (q + 1), :]
                        if sl % 2 == 0:
                            nc.scalar.activation(
                                dst, ps[:], AF.Identity, bias=bt[:, sl:sl + 1])
                        else:
                            nc.vector.tensor_scalar(
                                dst, ps[:], bt[:, sl:sl + 1], None, ALU.add)

            # ---------------- main loop ----------------
            with (
                tc.tile_pool(name="zps", bufs=3, space="PSUM") as zps,
                tc.tile_pool(name="eps", bufs=2, space="PSUM") as eps,
                tc.tile_pool(name="state", bufs=4) as state,
                tc.tile_pool(name="hpool", bufs=4) as hpool,
                tc.tile_pool(name="work", bufs=4) as work,
                tc.tile_pool(name="wsb", bufs=4) as wsb,
                tc.tile_pool(name="argp", bufs=4) as argp,
                tc.tile_pool(name="thp", bufs=4) as thp,
                tc.tile_pool(name="soft", bufs=2) as soft,
            ):
                H_of = {0: h0}
                gz_of = {}
                w_of = {}     # step -> wT sbuf tile (128, BS)
                arg_of = {}   # step -> ARG tile
                tanh_of = {}  # step -> TANH tile
                e_of = {}     # group -> E psum tile
                zt_of = {}    # step -> z psum tile (w region pending)

                # bootstrap: gz tile for step 0 holds c0 in its c-slot
                gz0 = state.tile([128, 160], BF, tag="gz")
                nc.sync.dma_start(out=gz0[:, 128:160], in_=c0t_d.ap())
                gz_of[0] = gz0

                for t in range(T + LAG_SM + 1):
                    tw = t - LAG_W
                    tt = t - LAG_TANH
                    te = t - LAG_E

                    if t <= T - 1:
                        # --- PE: eye-init + z matmuls of step t ---
                        zt = zps.tile([128, 144], F32, tag="zt")
                        zt_of[t] = zt
                        chain("pe", nc.tensor.matmul(
                            zt[:, 0:128], eye[:], xwt[:, :, t, :],
                            start=True, stop=False))
                        for sl in range(8):
                            for kc in range(2):
                                chain("pe", nc.tensor.matmul(
                                    zt[:, 16 * sl:16 * (sl + 1)],
                                    wht[:, kc, sl, :],
                                    H_of[t][:, 16 * kc:16 * (kc + 1)],
                                    start=False, stop=(kc == 1)))

                        gz = gz_of[t]
                        gz_next = state.tile([128, 160], BF, tag="gz")
                        gz_of[t + 1] = gz_next

                        # --- ACT: gates (one instr; g-weights pre-scaled) ---
                        chain("act", nc.scalar.activation(
                            gz[:, 0:128], zt[:, 0:128], AF.Tanh, scale=0.5))

                    # --- ACT: first tanh half of step t-LAG_TANH ---
                    if tt >= 0 and tt in arg_of:
                        th = thp.tile([128, N, BS], BF, tag="th")
                        tanh_of[tt] = th
                        chain("act", nc.scalar.activation(
                            th[:, 0:64, :], arg_of[tt][:, 0:64, :], AF.Tanh))

                    if t <= T - 1:
                        # --- DVE: fused LSTM update ---
                        # u = [(i~+1)*g~ | (f~+1)*c] = [2ig | 2fc]
                        u = work.tile([128, 64], BF, tag="u")
                        chain("dve", nc.vector.scalar_tensor_tensor(
                            u[:], gz[:, 0:64], 1.0, gz[:, 96:160],
                            ALU.add, ALU.mult))
                        s2 = work.tile([128, 32], F32, tag="s2")
                        chain("dve", nc.vector.tensor_tensor(
                            s2[:], u[:, 0:32], u[:, 32:64], ALU.add))
                        # c_new = 0.5*s2 -> c-slot of next gz
                        chain("dve", nc.vector.tensor_scalar(
                            gz_next[:, 128:160], s2[:], 0.5, None, ALU.mult))
                        # tanh(c_new) = tanh(0.5*s2)
                        tct = work.tile([128, 32], BF, tag="tc")
                        chain("act", nc.scalar.activation(
                            tct[:], s2[:], AF.Tanh, scale=0.5))
                        # H = (o~+1)*tc = 2h
                        Hn = hpool.tile([128, 32], BF, tag="H")
                        chain("dve", nc.vector.scalar_tensor_tensor(
                            Hn[:], gz[:, 64:96], 1.0, tct[:],
                            ALU.add, ALU.mult))
                        H_of[t + 1] = Hn
                        H_of.pop(t, None)
                        gz_of.pop(t, None)

                    # --- ACT: second tanh half of step t-LAG_TANH ---
                    if tt >= 0 and tt in arg_of:
                        th = tanh_of[tt]
                        chain("act", nc.scalar.activation(
                            th[:, 64:128, :], arg_of.pop(tt)[:, 64:128, :],
                            AF.Tanh))

                    # --- DVE: memset for a new e-accumulation group ---
                    if 0 <= te <= T - 1 and te % G == 0:
                        ge = te // G
                        e_of[ge] = eps.tile([128, N], F32, tag="eps",
                                            name="etile")
                        chain("dve", nc.vector.memset(e_of[ge][:], 0.0))

                    # --- DVE: broadcast w over n for step t-LAG_W ---
                    if 0 <= tw <= T - 1:
                        wt_sb = w_of.pop(tw)
                        arg = argp.tile([128, N, BS], BF, tag="arg")
                        wt_bc = bass.AP(
                            tensor=wt_sb.tensor, offset=wt_sb.offset,
                            ap=[wt_sb.ap[0], [0, N], [1, BS]])
                        chain("dve", nc.vector.tensor_tensor(
                            arg[:], uxt[:], wt_bc, ALU.add))
                        arg_of[tw] = arg

                    # --- PE: e-matmuls of step t-LAG_E ---
                    if 0 <= te <= T - 1:
                        se, ge = te % G, te // G
                        ep = e_of[ge]
                        th = tanh_of.pop(te)
                        for r in range(4):
                            for cg in range(4):
                                bb = 4 * cg + r
                                v = r * 8 + se
                                chain("pe", nc.tensor.matmul(
                                    ep[32 * cg:32 * (cg + 1), :],
                                    svd[:, v, :],
                                    th[:, :, bb],
                                    start=False,
                                    stop=(se == G - 1 and r == 3),
                                    tile_position=(0, 32 * cg),
                                ))

                    if t <= T - 1:
                        # --- PE: w^T matmuls (into ZWW region) ---
                        zt = zt_of[t]
                        hs = [Hn[:, 0:16], Hn[:, 16:32],
                              gz_next[:, 128:144], gz_next[:, 144:160]]
                        for kc in range(4):
                            chain("pe", nc.tensor.matmul(
                                zt[:, 128:144], wdt[:, kc, :], hs[kc],
                                start=False, stop=(kc == 3)))
                        wt_sb = wsb.tile([128, BS], BF, tag="wt")
                        chain("dve", nc.vector.tensor_copy(
                            wt_sb[:], zt[:, 128:144]))
                        w_of[t] = wt_sb
                        zt_of.pop(t, None)

                    # --- softmax + output of group (t-LAG_SM)//G ---
                    if t >= LAG_SM and (t - LAG_SM) % G == 0:
                        gs = (t - LAG_SM) // G
                        if gs in e_of:
                            ep = e_of.pop(gs)
                            ex = soft.tile([128, N], F32, tag="ex")
                            sm = soft.tile([128, 1], F32, tag="sm")
                            chain("act", nc.scalar.activation(
                                ex[:], ep[:], AF.Exp, accum_out=sm[:]))
                            rc = soft.tile([128, 1], F32, tag="rc")
                            chain("dve", nc.vector.reciprocal(rc[:], sm[:]))
                            # out = (exp * 1/sum) * X  in one fused op
                            ot = soft.tile([128, N], F32, tag="ot")
                            chain("dve", nc.vector.scalar_tensor_tensor(
                                ot[:], ex[:], rc[:, 0:1], xga[:, gs, :],
                                ALU.mult, ALU.mult))
                            nc.sync.dma_start(
                                out=out_d.ap()[:, G * gs:G * (gs + 1), :],
                                in_=ot[:])
    return nc


_CACHE = {}


def _get_nc():
    if "nc" not in _CACHE:
        nc = bacc.Bacc("TRN2", target_bir_lowering=False, debug=False)
        _build_kernel(nc)
        nc.compile()
        _CACHE["nc"] = nc
    return _CACHE["nc"]


def kernel(X, h0, s0, Wx, Wh, b, Wd, bW, Ud, bU, vd, bv):
    X = np.asarray(X, np.float32)
    h0 = np.asarray(h0, np.float32)
    s0 = np.asarray(s0, np.float32)
    Wx = np.asarray(Wx, np.float32)
    Wh = np.asarray(Wh, np.float32)
    b = np.asarray(b, np.float32)
    Wd = np.asarray(Wd, np.float32)
    bW = np.asarray(bW, np.float32)
    Ud = np.asarray(Ud, np.float32)
    bU = np.asarray(bU, np.float32)
    vd = np.asarray(vd, np.float32)

    # replicated (weight) marshalling — layout only, no FLOPs
    # Wh scaled by 0.5 (state H = 2h); g-gate slots re-doubled so the
    # single tanh(0.5*z) gate activation yields tanh(z_g) for g.
    wht = np.ascontiguousarray(
        Wh.reshape(2, 128, 8, 128).transpose(1, 0, 2, 3)[:, :, PERM, :]
    ).astype(np.float32) * 0.5
    wht[:, :, 6:8, :] *= 2.0
    wht = wht.astype(BF16)
    wxt = np.ascontiguousarray(
        Wx.reshape(128, 8, 128)[:, PERM, :]).astype(np.float32)
    wxt[:, 6:8, :] *= 2.0
    wxt = wxt.astype(BF16)
    udt = Ud.astype(BF16)
    # Wd h-half scaled by 0.5 (H = 2h); c-half unscaled
    wdt = np.ascontiguousarray(Wd.reshape(4, 128, 128).transpose(1, 0, 2)
                               ).astype(np.float32)
    wdt[:, 0:2, :] *= 0.5
    wdt = wdt.astype(BF16)
    svd = np.zeros((128, 32, 32), np.float32)
    for v in range(32):
        svd[:, v, v] = vd[:, 0]
    svd = svd.astype(BF16)
    bt = np.ascontiguousarray(b.reshape(8, 128)[PERM].T).astype(np.float32)
    bt[:, 6:8] *= 2.0
    bwc = bW.reshape(T, 1).astype(np.float32)
    buc = bU.reshape(T, 1).astype(np.float32)
    eye = np.eye(128, dtype=BF16)

    def tr_state(v):  # (16,256) -> (128, 32) with col = 16*j + b
        return np.ascontiguousarray(
            v.T.reshape(2, 128, BS).transpose(1, 0, 2).reshape(128, 2 * BS))

    in_maps = []
    for c in range(NCORES):
        xs = X[BS * c:BS * (c + 1)]
        in_maps.append({
            "x": np.ascontiguousarray(xs),
            "xnb": np.ascontiguousarray(xs.transpose(1, 0, 2)).astype(BF16),
            "xtb": np.ascontiguousarray(xs.transpose(2, 1, 0)).astype(BF16),
            "wht": wht, "wxt": wxt, "udt": udt, "wdt": wdt, "svd": svd,
            "bt": bt, "bwc": bwc, "buc": buc, "eye": eye,
            "h0t": (tr_state(h0[BS * c:BS * (c + 1)]) * 2.0).astype(BF16),
            "c0t": tr_state(s0[BS * c:BS * (c + 1)]).astype(BF16),
        })

    from concourse.bass_utils import run_bass_kernel_spmd
    nc = _get_nc()
    _CACHE["in_maps"] = in_maps
    res = run_bass_kernel_spmd(nc, in_maps, core_ids=list(range(NCORES)))
    out = np.concatenate(
        [np.asarray(res.results[c]["out"]) for c in range(NCORES)], axis=0)
    return out.astype(np.float32)


# revision 12
# speedup vs baseline: 1.8868x; 1.1219x over previous
"""Trainium2 Bass kernel for nn_EncoderInput (DA-RNN input-attention encoder).

Data-parallel over batch: 8 cores x 16 batch rows each. Full inputs in,
full output out; sharding/marshalling (transposes, bf16 casts, chunk
re-layout) happens host-side, all FLOPs happen on-device.

Per-core algorithm (B=16 shard, T=128 steps, N=128 drivers, M=256 hidden):
  UX[b,n,u]   = sum_t X[b,t,n] Ud[t,u] + bU[u] + bW[u]   (precomputed, PE)
  XW[t,m,b]   = (x_t @ Wx + b)^T                          (precomputed, PE)
  per step t (state: gz tile = [i~ f~ o~ g~ | c], H = 2h):
    z^T  = XW[t] + (Wh/2)^T-chunks @ H^T    (PE)
    gz   = tanh(z/2)  (i~,f~,o~ halves of sigmoid; g~ = tanh, w x2)  (ACT)
    u    = (gz[i,f]+1)*(gz[g],c) = [2ig | 2fc]            (DVE STT, 1 op)
    s2   = u_a + u_b = 2 c_new                            (DVE TT)
    c    = 0.5*s2 -> next gz c-slot                       (DVE TS)
    tc   = tanh(0.5*s2) = tanh(c_new)                     (ACT, scale trick)
    H    = (o~+1)*tc = 2h                                 (DVE STT)
    w^T  = (Wd-scaled)^T-chunks @ [H;c]^T                 (PE into z psum)
    arg  = UX^T + w^T[:,b] broadcast                      (DVE TT, lag 2)
    TANH = tanh(arg)  two [128,1024] halves packed around (ACT, lag 4)
          the recurrence's gates/tanh_c ops in the ACT queue
    e    = sparse-vd matmuls -> psum slots p=8b+s         (PE, lag 6)
  per 8-step group: exp(+accum sum) ; alpha*X fused STT ; DMA out

Every compute engine is given an explicit total order via no-sync
scheduling edges so the FIFO queues pack tightly without head-of-line
blocking of the recurrence by the big attention tanh.
"""

import sys

for _p in ("/opt/trn_rl_repo",):
    if _p not in sys.path:
        sys.path.insert(0, _p)

import numpy as np
import ml_dtypes

BF16 = ml_dtypes.bfloat16

import concourse.bass as bass
import concourse.tile as tile
from concourse.tile import add_dep_helper
from concourse import bacc, mybir

F32 = mybir.dt.float32
BF = mybir.dt.bfloat16
AF = mybir.ActivationFunctionType
ALU = mybir.AluOpType
AX = mybir.AxisListType

B, T, N, M = 128, 128, 128, 256
NCORES = 8
BS = B // NCORES          # 16 batch rows per core
G = 8                     # steps per softmax group
NG = T // G               # 16 groups
# psum slot -> m-chunk of z; layout becomes [i0 i1 f0 f1 o0 o1 g0 g1]
PERM = [0, 1, 2, 3, 6, 7, 4, 5]

# software-pipeline lags (in emission windows)
LAG_W = 2      # w-broadcast of step t emitted in window t+LAG_W
LAG_TANH = 4   # big tanh halves of step t in window t+LAG_TANH
LAG_E = 6      # e-matmuls of step t in window t+LAG_E
LAG_SM = 16    # softmax tail of group g at window 8g+LAG_SM


def _build_kernel(nc):
    x_d = nc.dram_tensor("x", [BS, T, N], F32, kind="ExternalInput")
    xnb_d = nc.dram_tensor("xnb", [T, BS, N], BF, kind="ExternalInput")
    xtb_d = nc.dram_tensor("xtb", [N, T, BS], BF, kind="ExternalInput")
    wht_d = nc.dram_tensor("wht", [128, 2, 8, 128], BF, kind="ExternalInput")
    wxt_d = nc.dram_tensor("wxt", [128, 8, 128], BF, kind="ExternalInput")
    udt_d = nc.dram_tensor("udt", [T, T], BF, kind="ExternalInput")
    wdt_d = nc.dram_tensor("wdt", [128, 4, T], BF, kind="ExternalInput")
    s_d = nc.dram_tensor("svd", [T, 32, 32], BF, kind="ExternalInput")
    bt_d = nc.dram_tensor("bt", [128, 8], F32, kind="ExternalInput")
    bwc_d = nc.dram_tensor("bwc", [T, 1], F32, kind="ExternalInput")
    buc_d = nc.dram_tensor("buc", [T, 1], F32, kind="ExternalInput")
    h0t_d = nc.dram_tensor("h0t", [128, 32], BF, kind="ExternalInput")
    c0t_d = nc.dram_tensor("c0t", [128, 32], BF, kind="ExternalInput")
    eye_d = nc.dram_tensor("eye", [128, 128], BF, kind="ExternalInput")
    out_d = nc.dram_tensor("out", [BS, T, N], F32, kind="ExternalOutput")

    # per-engine total-order chain (no-sync scheduling edges)
    last = {}

    def chain(key, instr):
        if key in last:
            add_dep_helper(instr.ins, last[key].ins, sync=False,
                           reason="engine order")
        last[key] = instr
        return instr

    with tile.TileContext(nc) as tc:
        with tc.tile_pool(name="const", bufs=1) as const:
            # persistent SBUF residents
            uxt = const.tile([128, N, BS], BF)         # [u, n, b]
            xwt = const.tile([128, 8, T, BS], BF)      # [m, slot, t, b]
            wht = const.tile([128, 2, 8, 128], BF)
            wxt = const.tile([128, 8, 128], BF)
            udt = const.tile([T, T], BF)
            wdt = const.tile([128, 4, T], BF)
            svd = const.tile([T, 32, 32], BF)
            bt = const.tile([128, 8], F32)
            bwc = const.tile([T, 1], F32)
            buc = const.tile([T, 1], F32)
            eye = const.tile([128, 128], BF)
            bubw = const.tile([T, 1], F32)
            xnb = const.tile([T, BS, N], BF)
            xtb = const.tile([N, T, BS], BF)
            xga = const.tile([128, 16, N], F32)
            h0 = const.tile([128, 32], BF)

            for sb, dr in [
                (wht, wht_d), (wxt, wxt_d), (udt, udt_d), (wdt, wdt_d),
                (svd, s_d), (bt, bt_d), (bwc, bwc_d), (buc, buc_d),
                (eye, eye_d), (xnb, xnb_d), (xtb, xtb_d),
                (h0, h0t_d),
            ]:
                nc.sync.dma_start(out=sb[:], in_=dr.ap())

            for bb in range(BS):
                src_ap = bass.AP(
                    tensor=x_d, offset=bb * T * N,
                    ap=[[N, 8], [8 * N, 16], [1, N]])
                nc.sync.dma_start(out=xga[8 * bb:8 * bb + 8, :, :],
                                  in_=src_ap)

            nc.vector.tensor_add(bubw[:], buc[:], bwc[:])

            # ---------------- precompute ----------------
            with tc.tile_pool(name="pre", bufs=2, space="PSUM") as pre:
                # UX^T: [u,(b,n)] = sum_t Ud[t,u] * X[b,t,n]  (+ bU + bW)
                for q in range(2):
                    ps = pre.tile([128, 1024], F32)
                    for r in range(2):
                        idx = 2 * q + r
                        nc.tensor.matmul(
                            ps[:, 512 * r:512 * (r + 1)],
                            udt[:],
                            xnb[:, 4 * idx:4 * (idx + 1), :],
                            start=True, stop=True,
                        )
                    uxt_dst = bass.AP(
                        tensor=uxt.tensor, offset=uxt.offset + 8 * q,
                        ap=[uxt.ap[0], [1, 8], [BS, N]])
                    nc.scalar.activation(
                        uxt_dst, ps[:].rearrange("p (b n) -> p b n", b=8),
                        AF.Identity, bias=bubw[:, 0:1],
                    )
                # XW^T: [m,(t,b)] = sum_n Wx[n,m] X[b,t,n]  (+ b)
                for sl in range(8):
                    ps = pre.tile([128, 1024], F32)
                    for q in range(2):
                        for r in range(2):
                            tq = 32 * (2 * q + r)
                            nc.tensor.matmul(
                                ps[:, 512 * r:512 * (r + 1)],
                                wxt[:, sl, :],
                                xtb[:, tq:tq + 32, :],
                                start=True, stop=True,
                            )
                        dst = xwt[:, sl, 64 * q:64 * (q + 1), :]
                        if sl % 2 == 0:
                            nc.scalar.activation(
                                dst, ps[:], AF.Identity, bias=bt[:, sl:sl + 1])
                        else:
                            nc.vector.tensor_scalar(
                                dst, ps[:], bt[:, sl:sl + 1], None, ALU.add)

            # ---------------- main loop ----------------
            with (
                tc.tile_pool(name="zps", bufs=3, space="PSUM") as zps,
                tc.tile_pool(name="eps", bufs=2, space="PSUM") as eps,
                tc.tile_pool(name="state", bufs=4) as state,
                tc.tile_pool(name="hpool", bufs=4) as hpool,
                tc.tile_pool(name="work", bufs=4) as work,
                tc.tile_pool(name="wsb", bufs=4) as wsb,
                tc.tile_pool(name="argp", bufs=4) as argp,
                tc.tile_pool(name="thp", bufs=4) as thp,
                tc.tile_pool(name="soft", bufs=2) as soft,
            ):
                H_of = {0: h0}
                gz_of = {}
                w_of = {}     # step -> wT sbuf tile (128, BS)
                arg_of = {}   # step -> ARG tile
                tanh_of = {}  # step -> TANH tile
                e_of = {}     # group -> E psum tile
                zt_of = {}    # step -> z psum tile (w region pending)

                # bootstrap: gz tile for step 0 holds c0 in its c-slot
                gz0 = state.tile([128, 160], BF, tag="gz")
                nc.sync.dma_start(out=gz0[:, 128:160], in_=c0t_d.ap())
                gz_of[0] = gz0

                def emit_z(step):
                    # eye-init + z matmuls for `step` (H_of[step] must exist)
                    zt = zps.tile([128, 144], F32, tag="zt")
                    zt_of[step] = zt
                    chain("pe", nc.tensor.matmul(
                        zt[:, 0:128], eye[:], xwt[:, :, step, :],
                        start=True, stop=False))
                    for sl in range(8):
                        for kc in range(2):
                            chain("pe", nc.tensor.matmul(
                                zt[:, 16 * sl:16 * (sl + 1)],
                                wht[:, kc, sl, :],
                                H_of[step][:, 16 * kc:16 * (kc + 1)],
                                start=False, stop=(kc == 1)))

                emit_z(0)

                for t in range(T + LAG_SM + 1):
                    tw = t - LAG_W
                    tt = t - LAG_TANH
                    te = t - LAG_E

                    # --- PE: e-matmuls of step t-LAG_E (inputs ready two
                    # windows ago -> keep them at the head of the PE queue) ---
                    if 0 <= te <= T - 1:
                        se, ge = te % G, te // G
                        ep = e_of[ge]
                        th = tanh_of.pop(te)
                        for r in range(4):
                            for cg in range(4):
                                bb = 4 * cg + r
                                v = r * 8 + se
                                chain("pe", nc.tensor.matmul(
                                    ep[32 * cg:32 * (cg + 1), :],
                                    svd[:, v, :],
                                    th[:, :, bb],
                                    start=False,
                                    stop=(se == G - 1 and r == 3),
                                    tile_position=(0, 32 * cg),
                                ))

                    if t <= T - 1:
                        gz = gz_of[t]
                        gz_next = state.tile([128, 160], BF, tag="gz")
                        gz_of[t + 1] = gz_next

                        # --- ACT: gates (one instr; g-weights pre-scaled) ---
                        chain("act", nc.scalar.activation(
                            gz[:, 0:128], zt_of[t][:, 0:128],
                            AF.Tanh, scale=0.5))

                    # --- ACT: first tanh half of step t-LAG_TANH ---
                    if tt >= 0 and tt in arg_of:
                        th = thp.tile([128, N, BS], BF, tag="th")
                        tanh_of[tt] = th
                        chain("act", nc.scalar.activation(
                            th[:, 0:64, :], arg_of[tt][:, 0:64, :], AF.Tanh))

                    # --- DVE: memset for next window's new e-group (early,
                    # so the group's first e-matmuls never wait on it) ---
                    tn = te + 1
                    if 0 <= tn <= T - 1 and tn % G == 0:
                        ge = tn // G
                        e_of[ge] = eps.tile([128, N], F32, tag="eps",
                                            name="etile")
                        chain("dve", nc.vector.memset(e_of[ge][:], 0.0))

                    if t <= T - 1:
                        # --- DVE: fused LSTM update ---
                        # u = [(i~+1)*g~ | (f~+1)*c] = [2ig | 2fc]
                        u = work.tile([128, 64], BF, tag="u")
                        chain("dve", nc.vector.scalar_tensor_tensor(
                            u[:], gz[:, 0:64], 1.0, gz[:, 96:160],
                            ALU.add, ALU.mult))
                        s2 = work.tile([128, 32], F32, tag="s2")
                        chain("dve", nc.vector.tensor_tensor(
                            s2[:], u[:, 0:32], u[:, 32:64], ALU.add))
                        # c_new = 0.5*s2 -> c-slot of next gz
                        chain("dve", nc.vector.tensor_scalar(
                            gz_next[:, 128:160], s2[:], 0.5, None, ALU.mult))
                        # tanh(c_new) = tanh(0.5*s2)
                        tct = work.tile([128, 32], BF, tag="tc")
                        chain("act", nc.scalar.activation(
                            tct[:], s2[:], AF.Tanh, scale=0.5))
                        # H = (o~+1)*tc = 2h
                        Hn = hpool.tile([128, 32], BF, tag="H")
                        chain("dve", nc.vector.scalar_tensor_tensor(
                            Hn[:], gz[:, 64:96], 1.0, tct[:],
                            ALU.add, ALU.mult))
                        H_of[t + 1] = Hn
                        H_of.pop(t, None)
                        gz_of.pop(t, None)

                    # --- ACT: second tanh half of step t-LAG_TANH ---
                    if tt >= 0 and tt in arg_of:
                        th = tanh_of[tt]
                        chain("act", nc.scalar.activation(
                            th[:, 64:128, :], arg_of.pop(tt)[:, 64:128, :],
                            AF.Tanh))

                    # --- DVE: broadcast w over n for step t-LAG_W ---
                    if 0 <= tw <= T - 1:
                        wt_sb = w_of.pop(tw)
                        arg = argp.tile([128, N, BS], BF, tag="arg")
                        wt_bc = bass.AP(
                            tensor=wt_sb.tensor, offset=wt_sb.offset,
                            ap=[wt_sb.ap[0], [0, N], [1, BS]])
                        chain("dve", nc.vector.tensor_tensor(
                            arg[:], uxt[:], wt_bc, ALU.add))
                        arg_of[tw] = arg

                    if t <= T - 2:
                        # --- PE: z matmuls of step t+1 (H just computed;
                        # emitting now keeps z ahead of next window's gates)
                        emit_z(t + 1)

                    if t <= T - 1:
                        # --- PE: w^T matmuls (into ZWW region) ---
                        zt = zt_of[t]
                        hs = [Hn[:, 0:16], Hn[:, 16:32],
                              gz_next[:, 128:144], gz_next[:, 144:160]]
                        for kc in range(4):
                            chain("pe", nc.tensor.matmul(
                                zt[:, 128:144], wdt[:, kc, :], hs[kc],
                                start=False, stop=(kc == 3)))
                        wt_sb = wsb.tile([128, BS], BF, tag="wt")
                        chain("dve", nc.vector.tensor_copy(
                            wt_sb[:], zt[:, 128:144]))
                        w_of[t] = wt_sb
                        zt_of.pop(t, None)

                    # --- softmax + output of group (t-LAG_SM)//G ---
                    if t >= LAG_SM and (t - LAG_SM) % G == 0:
                        gs = (t - LAG_SM) // G
                        if gs in e_of:
                            ep = e_of.pop(gs)
                            ex = soft.tile([128, N], F32, tag="ex")
                            sm = soft.tile([128, 1], F32, tag="sm")
                            chain("act", nc.scalar.activation(
                                ex[:], ep[:], AF.Exp, accum_out=sm[:]))
                            rc = soft.tile([128, 1], F32, tag="rc")
                            chain("dve", nc.vector.reciprocal(rc[:], sm[:]))
                            # out = (exp * 1/sum) * X  in one fused op
                            ot = soft.tile([128, N], F32, tag="ot")
                            chain("dve", nc.vector.scalar_tensor_tensor(
                                ot[:], ex[:], rc[:, 0:1], xga[:, gs, :],
                                ALU.mult, ALU.mult))
                            nc.sync.dma_start(
                                out=out_d.ap()[:, G * gs:G * (gs + 1), :],
                                in_=ot[:])
    return nc


_CACHE = {}


def _get_nc():
    if "nc" not in _CACHE:
        nc = bacc.Bacc("TRN2", target_bir_lowering=False, debug=False)
        _build_kernel(nc)
        nc.compile()
        _CACHE["nc"] = nc
    return _CACHE["nc"]


def kernel(X, h0, s0, Wx, Wh, b, Wd, bW, Ud, bU, vd, bv):
    X = np.asarray(X, np.float32)
    h0 = np.asarray(h0, np.float32)
    s0 = np.asarray(s0, np.float32)
    Wx = np.asarray(Wx, np.float32)
    Wh = np.asarray(Wh, np.float32)
    b = np.asarray(b, np.float32)
    Wd = np.asarray(Wd, np.float32)
    bW = np.asarray(bW, np.float32)
    Ud = np.asarray(Ud, np.float32)
    bU = np.asarray(bU, np.float32)
    vd = np.asarray(vd, np.float32)

    # replicated (weight) marshalling — layout only, no FLOPs
    # Wh scaled by 0.5 (state H = 2h); g-gate slots re-doubled so the
    # single tanh(0.5*z) gate activation yields tanh(z_g) for g.
    wht = np.ascontiguousarray(
        Wh.reshape(2, 128, 8, 128).transpose(1, 0, 2, 3)[:, :, PERM, :]
    ).astype(np.float32) * 0.5
    wht[:, :, 6:8, :] *= 2.0
    wht = wht.astype(BF16)
    wxt = np.ascontiguousarray(
        Wx.reshape(128, 8, 128)[:, PERM, :]).astype(np.float32)
    wxt[:, 6:8, :] *= 2.0
    wxt = wxt.astype(BF16)
    udt = Ud.astype(BF16)
    # Wd h-half scaled by 0.5 (H = 2h); c-half unscaled
    wdt = np.ascontiguousarray(Wd.reshape(4, 128, 128).transpose(1, 0, 2)
                               ).astype(np.float32)
    wdt[:, 0:2, :] *= 0.5
    wdt = wdt.astype(BF16)
    svd = np.zeros((128, 32, 32), np.float32)
    for v in range(32):
        svd[:, v, v] = vd[:, 0]
    svd = svd.astype(BF16)
    bt = np.ascontiguousarray(b.reshape(8, 128)[PERM].T).astype(np.float32)
    bt[:, 6:8] *= 2.0
    bwc = bW.reshape(T, 1).astype(np.float32)
    buc = bU.reshape(T, 1).astype(np.float32)
    eye = np.eye(128, dtype=BF16)

    def tr_state(v):  # (16,256) -> (128, 32) with col = 16*j + b
        return np.ascontiguousarray(
            v.T.reshape(2, 128, BS).transpose(1, 0, 2).reshape(128, 2 * BS))

    in_maps = []
    for c in range(NCORES):
        xs = X[BS * c:BS * (c + 1)]
        in_maps.append({
            "x": np.ascontiguousarray(xs),
            "xnb": np.ascontiguousarray(xs.transpose(1, 0, 2)).astype(BF16),
            "xtb": np.ascontiguousarray(xs.transpose(2, 1, 0)).astype(BF16),
            "wht": wht, "wxt": wxt, "udt": udt, "wdt": wdt, "svd": svd,
            "bt": bt, "bwc": bwc, "buc": buc, "eye": eye,
            "h0t": (tr_state(h0[BS * c:BS * (c + 1)]) * 2.0).astype(BF16),
            "c0t": tr_state(s0[BS * c:BS * (c + 1)]).astype(BF16),
        })

    from concourse.bass_utils import run_bass_kernel_spmd
    nc = _get_nc()
    _CACHE["in_maps"] = in_maps
    res = run_bass_kernel_spmd(nc, in_maps, core_ids=list(range(NCORES)))
    out = np.concatenate(
        [np.asarray(res.results[c]["out"]) for c in range(NCORES)], axis=0)
    return out.astype(np.float32)
